# revision 11
# baseline (speedup 1.0000x reference)
"""minLSTM (2-layer, B=4, S=4096, D=1024) on 8 Trainium2 NeuronCores.

Sharding: core k -> (batch b = k//2, channel half h = k%2).
Each core computes all 4096 timesteps for its batch and its 512 channels:
  - gates via PE matmuls in bf16 (lhsT = W^T shard, rhs = x^T), laid out
    (gate-channel partition x token free) so the recurrence layout is native,
  - normalized gates f' = sig(f)/(sig(f)+sig(i)) via ACT sigmoids + one DVE
    reciprocal, with the cheap elementwise ops (ssum, g-max, h-mult) offloaded
    to the otherwise-idle Pool engine,
  - g = max(cell+b+0.5, sig(cell+b)),
  - b~ = (f'-1)*g via one fused scalar_tensor_tensor, then the recurrence
    c_t = f'*c_{t-1} + (1-f')*g_t as tensor_tensor_scan(mult, subtract),
  - h = sig(o) * c.
x-tile loads are prefetched 3 (block,layer) units ahead so they never queue
behind h-stores on the in-order SP DMA queue. Between the two layers,
channel-half pairs exchange h1 (bf16) via pairwise AllGather collectives.

Self-contained: hardcodes shapes; only imports the system concourse repo.
"""
import sys

if '/opt/trn_rl_repo' not in sys.path:
    sys.path.insert(0, '/opt/trn_rl_repo')

import numpy as np

B, S, D = 4, 4096, 1024
NCORES = 8
HALF = D // 2           # channels per core: 512
NCHUNK = HALF // 128    # 4 partition chunks of 128 channels
NKT = D // 128          # 8 contraction k-tiles
TBLK = 512              # token block
NBLK = S // TBLK        # 8 token blocks
GCH = 4 * HALF          # gate channels per core: 2048
PF = 3                  # x-tile prefetch depth in (layer, block) units

_CACHE = {}


def _split_multi_waits(nc):
    """This walrus build rejects >1 sync wait per instruction. Hoist extra
    waits onto same-engine NoOps inserted just before; engine-queue program
    order makes this semantically identical."""
    from concourse import mybir
    n = 0
    for fn in nc.m.functions:
        for blk in fn.blocks:
            insts = list(blk.instructions)
            new = []
            changed = False
            for inst in insts:
                si = inst.sync_info
                ow = list(si.on_wait) if si is not None and si.on_wait else []
                if len(ow) > 1:
                    changed = True
                    for w in ow[:-1]:
                        n += 1
                        nop = mybir.InstNoOp(name=f"I-wsplit-{n}", ins=[], outs=[])
                        nop.engine = inst.engine
                        nop.sync_info = mybir.SyncInfo(on_wait=[w], on_update=[])
                        new.append(nop)
                    si.on_wait = [ow[-1]]
                new.append(inst)
            if changed:
                blk.instructions = new
    return n


def _build_nc(mm_mode="bf16"):
    import concourse.bass as bass
    import concourse.mybir as mybir
    import concourse.tile as tile

    f32 = mybir.dt.float32
    fmm = {"f32r": mybir.dt.float32r, "f32": f32,
           "bf16": mybir.dt.bfloat16}[mm_mode]
    fh1 = mybir.dt.bfloat16 if mm_mode == "bf16" else f32
    AF = mybir.ActivationFunctionType
    ALU = mybir.AluOpType

    nc = bass.Bass("TRN2", target_bir_lowering=False, debug=False,
                   num_devices=NCORES)

    xT_d = nc.dram_tensor("xT", [D, S], fmm, kind="ExternalInput").ap()
    w_d = [nc.dram_tensor(f"w{l}t", [D, GCH], fmm, kind="ExternalInput").ap()
           for l in range(2)]
    ba_d = [nc.dram_tensor(f"b{l}a", [128, 16], f32, kind="ExternalInput").ap()
            for l in range(2)]
    bc_d = [nc.dram_tensor(f"b{l}c", [128, 4], f32, kind="ExternalInput").ap()
            for l in range(2)]
    cp_d = [nc.dram_tensor(f"cp{l}", [128, 4], f32, kind="ExternalInput").ap()
            for l in range(2)]
    h2t_d = nc.dram_tensor("h2t", [HALF, S], f32, kind="ExternalOutput").ap()

    with tile.TileContext(nc) as tc:
        with tc.tile_pool(name="wp", bufs=1) as wp, \
             tc.tile_pool(name="xkp", bufs=PF + 1) as xkp, \
             tc.tile_pool(name="gp", bufs=2) as gp, \
             tc.tile_pool(name="cgp", bufs=3) as cgp, \
             tc.tile_pool(name="cp", bufs=1) as cpool, \
             tc.tile_pool(name="psum", bufs=8, space="PSUM") as psum, \
             tc.tile_pool(name="dstage", bufs=2, space="DRAM") as dstage, \
             tc.tile_pool(name="dfull", bufs=8, space="DRAM") as dfull:

            # h1 gathered blocks must persist through layer 2: 8 live tiles
            h1f = [dfull.tile([D, TBLK], fh1, tag="h1f", name=f"h1f{t}")
                   for t in range(NBLK)]

            # Weight layout (host side): gate-channel index ct = j*4 + q so a
            # chunk j's four gate slices are one contiguous [128,512] span.
            # Layer-0 weights stream per (chunk, k) on the SP queue right
            # behind the first x block; layer-1 weights go on the ACT hwdge
            # queue so they never delay layer-0's pipeline.
            w_ks = {}
            ba = {}
            bc = {}
            cp = {}
            for l in range(2):
                ba[l] = cpool.tile([128, 16], f32, tag=f"ba{l}", name=f"ba{l}")
                nc.sync.dma_start(ba[l][:], ba_d[l][:])
                bc[l] = cpool.tile([128, 4], f32, tag=f"bc{l}", name=f"bc{l}")
                nc.sync.dma_start(bc[l][:], bc_d[l][:])
                cp[l] = cpool.tile([128, 4], f32, tag=f"cp{l}", name=f"cp{l}")
                nc.sync.dma_start(cp[l][:], cp_d[l][:])
                w_ks[l] = [wp.tile([128, GCH], fmm, tag=f"Wk{l}_{k}",
                                   name=f"w{l}_{k}") for k in range(NKT)]

            units = [(l, t) for l in range(2) for t in range(NBLK)]
            xk_tiles = {}

            def load_unit(u):
                l, t = units[u]
                xk_ks = []
                for k in range(NKT):
                    xkt = xkp.tile([128, TBLK], fmm, tag=f"xk{k}",
                                   name=f"xk{l}_{t}_{k}")
                    if l == 0:
                        src = xT_d[k * 128:(k + 1) * 128,
                                   t * TBLK:(t + 1) * TBLK]
                    else:
                        src = h1f[t][k * 128:(k + 1) * 128, :]
                    nc.sync.dma_start(
                        xkt[:],
                        src if src.dtype == fmm else src.bitcast(fmm))
                    xk_ks.append(xkt)
                xk_tiles[u] = xk_ks

            load_unit(0)
            for j in range(NCHUNK):
                for k in range(NKT):
                    nc.sync.dma_start(
                        w_ks[0][k][:, j * 512:(j + 1) * 512],
                        w_d[0][k * 128:(k + 1) * 128, j * 512:(j + 1) * 512])
            for u in range(1, min(PF, len(units))):
                load_unit(u)
            for k in range(NKT):
                nc.scalar.dma_start(w_ks[1][k][:], w_d[1][k * 128:(k + 1) * 128, :])

            carry = {0: [None] * NCHUNK, 1: [None] * NCHUNK}
            for u, (l, t) in enumerate(units):
                if u + PF < len(units):
                    load_unit(u + PF)
                xk_ks = xk_tiles.pop(u)

                if l == 0:
                    h1own = dstage.tile([HALF, TBLK], fh1, tag="h1own",
                                        name=f"h1own{t}")

                for j in range(NCHUNK):
                    ps = {}
                    for q in ("i", "f", "o", "cell"):
                        ps[q] = psum.tile([128, TBLK], f32, tag="ps",
                                          name=f"ps_{q}{l}_{t}_{j}")
                    # k-outer so each weight k-tile is consumed as it arrives
                    for k in range(NKT):
                        for qi, q in enumerate(("i", "f", "o", "cell")):
                            ct = j * 4 + qi
                            nc.tensor.matmul(
                                ps[q][:],
                                w_ks[l][k][:, ct * 128:(ct + 1) * 128],
                                xk_ks[k][:],
                                start=(k == 0), stop=(k == NKT - 1))

                    def col(qi_):
                        return ba[l][:, j * 4 + qi_:j * 4 + qi_ + 1]

                    # ACT: four sigmoids + biased cell (order feeds DVE chain)
                    sf = gp.tile([128, TBLK], f32, tag="sf", name=f"sf{l}{t}{j}")
                    nc.scalar.activation(sf[:], ps["f"][:], AF.Sigmoid,
                                         bias=col(1))
                    si = gp.tile([128, TBLK], f32, tag="si", name=f"si{l}{t}{j}")
                    nc.scalar.activation(si[:], ps["i"][:], AF.Sigmoid,
                                         bias=col(0))
                    sg = gp.tile([128, TBLK], f32, tag="sg", name=f"sg{l}{t}{j}")
                    nc.scalar.activation(sg[:], ps["cell"][:], AF.Sigmoid,
                                         bias=col(3))
                    # cp5 on ACT (not a DVE STT from PSUM) so the cell PSUM
                    # bank is freed as soon as ACT drains it — the DVE is
                    # otherwise ~3.3us behind on the reciprocal and PE stalls
                    # waiting for PSUM banks.
                    cp5 = gp.tile([128, TBLK], f32, tag="cp5",
                                  name=f"cq{l}{t}{j}")
                    nc.scalar.activation(cp5[:], ps["cell"][:], AF.Identity,
                                         bias=bc[l][:, j:j + 1])
                    so = gp.tile([128, TBLK], f32, tag="so", name=f"so{l}{t}{j}")
                    nc.scalar.activation(so[:], ps["o"][:], AF.Sigmoid,
                                         bias=col(2))

                    # Pool: ssum = sf + si;  DVE: r = 1/ssum;  Pool: a = sf*r
                    ssum = gp.tile([128, TBLK], f32, tag="ssum",
                                   name=f"ss{l}{t}{j}")
                    nc.gpsimd.tensor_tensor(ssum[:], sf[:], si[:], ALU.add)
                    r = gp.tile([128, TBLK], f32, tag="r", name=f"r{l}{t}{j}")
                    nc.vector.reciprocal(r[:], ssum[:])
                    a = gp.tile([128, TBLK], f32, tag="a", name=f"a{l}{t}{j}")
                    nc.gpsimd.tensor_tensor(a[:], sf[:], r[:], ALU.mult)
                    # DVE: g = max(cell + bc + 0.5, sig(cell + b))
                    g = gp.tile([128, TBLK], f32, tag="g", name=f"g{l}{t}{j}")
                    nc.vector.tensor_tensor(g[:], cp5[:], sg[:], ALU.max)
                    # DVE: btn = (a - 1) * g  (scan's subtract adds (1-a)*g)
                    btn = gp.tile([128, TBLK], f32, tag="btn",
                                  name=f"bt{l}{t}{j}")
                    nc.vector.scalar_tensor_tensor(
                        btn[:], a[:], 1.0, g[:], ALU.subtract, ALU.mult)
                    c = cgp.tile([128, TBLK], f32, tag=f"c{j}",
                                 name=f"c{l}{t}{j}")
                    init = cp[l][:, j:j + 1] if t == 0 else carry[l][j]
                    nc.vector.tensor_tensor_scan(c[:], a[:], btn[:], init,
                                                 ALU.mult, ALU.subtract)
                    carry[l][j] = c[:, TBLK - 1:TBLK]
                    hdt = fh1 if l == 0 else f32
                    h = gp.tile([128, TBLK], hdt, tag=f"h{l}",
                                name=f"h{l}{t}{j}")
                    nc.gpsimd.tensor_tensor(h[:], so[:], c[:], ALU.mult)

                    if l == 0:
                        nc.sync.dma_start(
                            h1own[j * 128:(j + 1) * 128, :], h[:])
                    else:
                        nc.sync.dma_start(
                            h2t_d[j * 128:(j + 1) * 128,
                                  t * TBLK:(t + 1) * TBLK], h[:])

                if l == 0:
                    nc.gpsimd.collective_compute(
                        "AllGather", ALU.bypass,
                        replica_groups=[[0, 1], [2, 3], [4, 5], [6, 7]],
                        ins=[h1own.opt()],
                        outs=[h1f[t].opt()],
                    )

    _split_multi_waits(nc)
    return nc


def _shard_inputs(x, W0, b0, W1, b1, c0_prev, c1_prev, mm_mode="bf16"):
    import ml_dtypes
    mmdt = ml_dtypes.bfloat16 if mm_mode == "bf16" else np.float32
    x = np.asarray(x, dtype=np.float32)
    in_maps = []
    xT = [np.ascontiguousarray(x[b].T.astype(mmdt)) for b in range(B)]
    per_layer = []
    for (W, bb) in ((W0, b0), (W1, b1)):
        W = np.asarray(W, dtype=np.float32)
        bb = np.asarray(bb, dtype=np.float32)
        halves = []
        for h in range(2):
            # gate-channel order ct = j*4 + q (chunk-major) so each chunk's
            # four gate weight slices are one contiguous [*, 512] span
            rows = np.concatenate(
                [q * D + h * HALF + j * 128 + np.arange(128)
                 for j in range(4) for q in range(4)])
            wt = np.ascontiguousarray(W[rows, :].T.astype(mmdt))  # (D, GCH)
            ba = np.ascontiguousarray(bb[rows].reshape(16, 128).T)  # (128,16)
            bc = np.ascontiguousarray(
                ba[:, 3::4] + np.float32(0.5))  # cell cols (ct=j*4+3)
            halves.append((wt, ba, bc))
        per_layer.append(halves)
    cps = []
    for cprev in (c0_prev, c1_prev):
        cprev = np.asarray(cprev, dtype=np.float32)
        halves = []
        for b in range(B):
            row = []
            for h in range(2):
                seg = cprev[b, 0, h * HALF:(h + 1) * HALF]
                row.append(np.ascontiguousarray(seg.reshape(4, 128).T))
            halves.append(row)
        cps.append(halves)
    for k in range(NCORES):
        b, h = k // 2, k % 2
        m = {"xT": xT[b]}
        for l in range(2):
            wt, ba, bc = per_layer[l][h]
            m[f"w{l}t"] = wt
            m[f"b{l}a"] = ba
            m[f"b{l}c"] = bc
            m[f"cp{l}"] = cps[l][b][h]
        in_maps.append(m)
    return in_maps


import os
MM_MODE = os.environ.get("MINLSTM_MM_MODE", "bf16")


def _get_nc():
    if "nc" not in _CACHE:
        _CACHE["nc"] = _build_nc(mm_mode=MM_MODE)
    return _CACHE["nc"]


def kernel(x, W0, b0, W1, b1, c0_prev, c1_prev):
    from concourse.bass_utils import run_bass_kernel_spmd

    nc = _get_nc()
    in_maps = _shard_inputs(x, W0, b0, W1, b1, c0_prev, c1_prev, MM_MODE)
    res = run_bass_kernel_spmd(nc, in_maps, list(range(NCORES)))
    out = np.empty((B, S, D), dtype=np.float32)
    for k in range(NCORES):
        b, h = k // 2, k % 2
        out[b, :, h * HALF:(h + 1) * HALF] = res.results[k]["h2t"].T
    return out
